# revision 23
# baseline (speedup 1.0000x reference)
"""CPAMDec attention-decoder kernel for 8 Trainium2 NeuronCores.

Reference computation (per batch n of N=8):
    q  = x_n^T @ wq^T + bq          (HW=4096, C4=128)
    k  = y_n @ wk^T + bk            (K=32, C4=128)
    v  = y_n @ wv^T + bv            (K=32, C=512)
    attn = softmax(q @ k^T, axis=-1)        (HW, K)
    out = scale * (v^T @ attn^T) + x_n      (C, HW)

Sharding: pure data parallel - core i computes batch i.

This version is DMA-traffic-optimized: the tolerance (2e-2) leaves room
to ship x, out and all params as fp16, halving HBM bytes (the dominant
cost: the fp32 version is DMA-saturated for its entire 80us runtime).
Host pre-arranges x and out in partition-major layout so every bulk DMA
line is 4KB contiguous (128 descriptors per 512KB chunk instead of 512).

All 8 input chunks are prefetched up front; since the PE's work (~17us
at full clock) exceeds the per-chunk DMA cadence, the PE stream stays
dense once started, which keeps the HAM clock gate at 2.4 GHz (any PE
idle gap resets the clock to 0.65 GHz - the fp32 baseline lost 2x here).

Bias folding:
  - bq contributes a per-key bias e_b[j] = sum_o bq[o]*k'[j,o], applied
    inside the exp() activation (exact algebra); a constant -6 shift is
    folded in as well so exp() stays in fp16 range (softmax-invariant).
  - bv is folded into v via an extra rank-1 matmul row, and scale s into
    v_sb = s*(v+bv), so the output stage is a plain residual add.
"""

import sys

sys.path.insert(0, "/opt/trn_rl_repo")

import numpy as np

import concourse.bacc as bacc
import concourse.mybir as mybir
import concourse.tile as tile
from concourse.bass_utils import run_bass_kernel_spmd

F32 = mybir.dt.float32
F16 = mybir.dt.float16
AF = mybir.ActivationFunctionType

N, C, H, W, K = 8, 512, 64, 64, 32
HW = H * W            # 4096
C4 = C // 4           # 128
PC = 512              # free-dim chunk (1 PSUM bank of fp32)
NPC = HW // PC        # 8 chunks
KC = C // 128         # 4 contraction chunks
CT = C // 128         # 4 output row-tiles
CW = KC * PC          # 2048 elements per chunk per partition
PKW = 12 * 128 + C + 2  # packed-const tile width (wq|yt|wkt|bv|bq|bk)
ESHIFT = -6.0         # exp shift: keeps exp() outputs in fp16 range
WARMUP = 3            # PE busy-bridge matmuls (preamble-end -> first q)


def _emit(nc, tc):
    sync = nc.sync

    with (
        tc.tile_pool(name="const", bufs=1) as cst,
        tc.tile_pool(name="xbuf", bufs=1) as xp,
        tc.tile_pool(name="work", bufs=3) as wk_pool,
        tc.tile_pool(name="ps", bufs=2, space="PSUM") as ps,
    ):
        # ---- constant loads (scalar ring — just 2 triggers, then the
        # ACT queue is free for compute). pk packs wq(4)/yt(4)/wkt(4)
        # [128,128] tiles plus the replicated bv row and the bq/bk
        # columns, so there are no tiny-descriptor const DMAs (a [C4,1]
        # fp32 load is 128 4-byte descriptors that straggle for ~10us
        # behind the bulk streams).
        pk = cst.tile([128, PKW], F16, name="pk", tag="pk")
        nc.scalar.dma_start(pk[:], nc.t.pk[:])
        wvp = cst.tile([128, KC * C], F16, name="wvp", tag="wvp")
        nc.scalar.dma_start(wvp[:], nc.t.wvp[:])
        s_bc32 = cst.tile([K, 1], F32, name="s_bc32", tag="s_bc32")
        nc.gpsimd.dma_start(
            s_bc32[:], nc.t.s[:].partition_broadcast(K).squeeze(-1))

        def wq_t(k):
            return pk[:, k * 128:(k + 1) * 128]

        def yt_t(k):
            return pk[:, (4 + k) * 128:(5 + k) * 128]

        def wkt_t(k):
            return pk[:, (8 + k) * 128:(9 + k) * 128]

        bv_mov = pk[0:1, 1536:1536 + C]     # [1, C] bv row (partition 0)
        bq_col = pk[:, PKW - 2:PKW - 1]     # [C4, 1] bq column
        bk_col = pk[:, PKW - 1:PKW]         # [C4, 1] bk column

        def wv_t(k):
            return wvp[:, k * C:(k + 1) * C]

        # memset-backed constants (no DMA dependency -> early warm-up)
        ones32 = cst.tile([K, 128], F16, name="ones32", tag="ones32")
        nc.gpsimd.memset(ones32[:], 1.0)
        onesk = cst.tile([1, K], F16, name="onesk", tag="onesk")
        nc.gpsimd.memset(onesk[:], 1.0)
        dmy_m = cst.tile([K, PC], F16, name="dmy_m", tag="dmy_m")
        nc.gpsimd.memset(dmy_m[:], 0.0)

        # ---- x prefetch: all up front on the sync ring (store triggers
        # queue behind them). xs[pc] = (k01_ap, k23_ap) half-chunk APs;
        # chunk 0 loads as two halves so q(0) can start on the first
        # 256KB; the rest in 1MB pairs to cut trigger count.
        HF = CW // 2
        xs = [None] * NPC

        def _halves(t, off):
            return (t[:, off:off + HF], t[:, off + HF:off + CW])

        x0a = xp.tile([128, HF], F16, name="x0a", tag="x0a")
        sync.dma_start(x0a[:], nc.t.x16[:, 0:HF])
        x0b = xp.tile([128, HF], F16, name="x0b", tag="x0b")
        sync.dma_start(x0b[:], nc.t.x16[:, HF:CW])
        xs[0] = (x0a[:], x0b[:])
        x1 = xp.tile([128, CW], F16, name="x1", tag="x1")
        sync.dma_start(x1[:], nc.t.x16[:, CW:2 * CW])
        xs[1] = _halves(x1, 0)
        for pc in (2, 4, 6):
            t = xp.tile([128, 2 * CW], F16, name=f"xs{pc}", tag=f"xs{pc}")
            sync.dma_start(t[:], nc.t.x16[:, pc * CW:(pc + 2) * CW])
            xs[pc] = _halves(t, 0)
            xs[pc + 1] = _halves(t, CW)

        # ---- PE warm-up: HAM clock gate needs ~3.4us of sustained ----
        # matmul activity to unthrottle 0.65 -> 2.4 GHz.
        dmy_ps = ps.tile([128, PC], F32, name="dmy_ps", tag="q", bufs=2)
        for _ in range(WARMUP):
            nc.tensor.matmul(dmy_ps[:], ones32[:], dmy_m[:],
                             start=True, stop=True)

        pro = {}

        def emit_kt():
            # kT (with bk) — needs only pk, so it fills the x0 wait.
            kt_ps = ps.tile([C4, 4 * K], F32, name="kt_ps", tag="e", bufs=1)
            for k in range(KC):
                nc.tensor.matmul(kt_ps[:], wkt_t(k), yt_t(k),
                                 start=(k == 0), stop=(k == KC - 1))
            ktb4 = cst.tile([C4, 4 * K], F16, name="ktb4", tag="ktb4")
            nc.scalar.activation(out=ktb4[:], in_=kt_ps[:], func=AF.Identity,
                                 bias=bk_col, scale=1.0)
            pro['ktb4'] = ktb4

        def emit_eb():
            eb_ps = ps.tile([4 * K, 1], F32, name="eb_ps", tag="o23",
                            bufs=1)
            nc.tensor.matmul(eb_ps[:], pro['ktb4'][:], bq_col, start=True,
                             stop=True)
            e_b4 = cst.tile([4 * K, 1], F32, name="e_b4", tag="e_b4")
            nc.scalar.activation(out=e_b4[:], in_=eb_ps[:],
                                 func=AF.Copy, bias=ESHIFT, scale=1.0)
            pro['e_b4'] = e_b4

        def emit_v():
            v_ps = ps.tile([K, C], F32, name="v_ps", tag="s", bufs=1)
            for k in range(KC):
                nc.tensor.matmul(v_ps[:], yt_t(k)[:, 0:K], wv_t(k),
                                 start=(k == 0), stop=False)
            # rank-1 bias row: v += 1 * bv  (exact)
            nc.tensor.matmul(v_ps[:], onesk[:], bv_mov,
                             start=False, stop=True)
            v_sb = cst.tile([K, C], F16, name="v_sb", tag="v_sb")
            nc.scalar.activation(out=v_sb[:], in_=v_ps[:], func=AF.Copy,
                                 bias=0.0, scale=s_bc32[:])
            # partition-stacked copy for row-packed final matmuls:
            # vstack[32*ct + j, m] = v_sb[j, 128*ct + m]
            vstack = cst.tile([128, 128], F16, name="vstack", tag="vstack")
            for ct in range(CT):
                nc.gpsimd.dma_start(
                    vstack[32 * ct:32 * (ct + 1), :],
                    v_sb[:, 128 * ct:128 * (ct + 1)])
            pro['vstack'] = vstack

        # ------------- software-pipelined main loop over column chunks
        #   step:   q(step)   e/exp(step-1)   sum/rec/mul(step-2)
        #           finals/add/store(step-3)
        qtcs = [None] * NPC
        expts = [None] * NPC
        attns = [None] * NPC

        def stage_q(pc):
            k01, k23 = xs[pc]
            q_ps = ps.tile([C4, PC], F32, name=f"q_ps{pc}", tag="q", bufs=2)
            for k in range(KC):
                src = k01 if k < 2 else k23
                nc.tensor.matmul(q_ps[:], wq_t(k),
                                 src[:, (k % 2) * PC:(k % 2 + 1) * PC],
                                 start=(k == 0), stop=(k == KC - 1))
            qtc = wk_pool.tile([C4, PC], F16, name="qtc", tag="qtc", bufs=4)
            nc.scalar.activation(out=qtc[:], in_=q_ps[:], func=AF.Copy,
                                 scale=1.0)
            qtcs[pc] = qtc

        def stage_energy(pc):
            e_ps = ps.tile([128, PC], F32, name=f"e_ps{pc}", tag="e", bufs=1)
            nc.tensor.matmul(e_ps[:], pro['ktb4'][:], qtcs[pc][:],
                             start=True, stop=True)
            expt = wk_pool.tile([128, PC], F16, name="expt", tag="expt",
                                bufs=4)
            nc.scalar.activation(out=expt[:], in_=e_ps[:], func=AF.Exp,
                                 bias=pro['e_b4'][:], scale=1.0)
            expts[pc] = expt

        def stage_softmax(pc):
            s_ps = ps.tile([128, PC], F32, name=f"s_ps{pc}", tag="s", bufs=1)
            nc.tensor.matmul(s_ps[:], ones32[:], expts[pc][0:K, :],
                             start=True, stop=True)
            rec = wk_pool.tile([128, PC], F32, name="rec", tag="rec", bufs=4)
            nc.vector.reciprocal_approx_fast(out=rec[:], in_=s_ps[:])
            attn = wk_pool.tile([128, PC], F16, name="attn", tag="attn",
                                bufs=4)
            nc.vector.tensor_mul(attn[:], expts[pc][:], rec[:])
            attns[pc] = attn

        def stage_out(pc):
            k01, k23 = xs[pc]
            attn = attns[pc]
            vst = pro['vstack']
            osb = wk_pool.tile([128, CT * PC], F16, name="osb", tag="osb",
                               bufs=3)
            half = 2 * PC
            # ct pair 2,3 first: ACT downcast + Pool add (slow path);
            # each pair shares one 2-bank PSUM tile -> one wide op.
            o23 = ps.tile([128, 2 * PC], F32, name=f"o23_{pc}", tag="o23",
                          bufs=1)
            for ct in (2, 3):
                nc.tensor.matmul(o23[:, (ct - 2) * PC:(ct - 1) * PC],
                                 vst[32 * ct:32 * (ct + 1), :],
                                 attn[32 * ct:32 * (ct + 1), :],
                                 start=True, stop=True,
                                 tile_position=(32 * ct, 0))
            oc = wk_pool.tile([128, 2 * PC], F16, name="oc", tag="oc",
                              bufs=2)
            nc.scalar.activation(out=oc[:], in_=o23[:], func=AF.Copy,
                                 scale=1.0)
            nc.gpsimd.tensor_add(osb[:, half:], oc[:], k23[:])
            sync.dma_start(nc.t.out16[:, pc * CW + half:(pc + 1) * CW],
                           osb[:, half:])

            o01 = ps.tile([128, 2 * PC], F32, name=f"o01_{pc}", tag="o01",
                          bufs=1)
            for ct in (0, 1):
                nc.tensor.matmul(o01[:, ct * PC:(ct + 1) * PC],
                                 vst[32 * ct:32 * (ct + 1), :],
                                 attn[32 * ct:32 * (ct + 1), :],
                                 start=True, stop=True,
                                 tile_position=(32 * ct, 0))
            nc.vector.tensor_add(osb[:, 0:half], o01[:], k01[:])
            sync.dma_start(nc.t.out16[:, pc * CW:pc * CW + half],
                           osb[:, 0:half])

        # hand-rolled ramp: prioritize chunk 0's chain end to end
        emit_kt()
        stage_q(0)
        emit_eb()
        stage_energy(0)
        stage_q(1)
        stage_softmax(0)
        emit_v()
        stage_q(2)
        stage_energy(1)
        stage_out(0)
        stage_softmax(1)
        for step in range(3, NPC + 3):
            if 0 <= step - 3 < NPC:
                stage_out(step - 3)
            if step < NPC:
                stage_q(step)
            if 0 <= step - 1 < NPC:
                stage_energy(step - 1)
            if 0 <= step - 2 < NPC:
                stage_softmax(step - 2)


class _T:
    """Attribute access to declared dram params."""
    def __init__(self):
        self.__dict__ = {}


_NC_CACHE = []


def _build():
    if _NC_CACHE:
        return _NC_CACHE[0]
    nc = bacc.Bacc(target_bir_lowering=False)
    nc.t = _T()
    t = nc.t
    t.x16 = nc.declare_dram_parameter("x16", [128, NPC * CW], F16,
                                      isOutput=False)
    t.pk = nc.declare_dram_parameter("pk", [128, PKW], F16,
                                     isOutput=False)
    t.wvp = nc.declare_dram_parameter("wvp", [128, KC * C], F16,
                                      isOutput=False)
    t.s = nc.declare_dram_parameter("s", [1, 1], F32, isOutput=False)
    t.out16 = nc.declare_dram_parameter("out16", [128, NPC * CW], F16,
                                        isOutput=True)
    with tile.TileContext(nc) as tc:
        _emit(nc, tc)
    nc.finalize()
    _NC_CACHE.append(nc)
    return nc


def _in_maps(x, y, wq, bq, wk, bk, wv, bv, scale):
    f16 = np.float16
    # x: (N,C,H,W) -> per-core [128, NPC*KC*PC] partition-major fp16,
    # so every chunk DMA line is 4KB contiguous per partition.
    x16 = (np.asarray(x, dtype=np.float32)
           .reshape(N, KC, 128, NPC, PC)
           .transpose(0, 2, 3, 1, 4)
           .reshape(N, 128, NPC * CW)
           .astype(f16))
    # packed const tile: wq(4)|yt(4)|wkt(4) [128,128] tiles along free
    # dim, then the bv row replicated across partitions and the bq/bk
    # columns.
    wq_p = np.float32(wq).T.reshape(KC, 128, C4).transpose(1, 0, 2)
    wq_p = wq_p.reshape(128, 4 * C4)
    yt_p = (np.tile(np.transpose(np.float32(y), (0, 2, 1)), (1, 1, 4))
            .reshape(N, KC, 128, 4 * K).transpose(0, 2, 1, 3)
            .reshape(N, 128, 4 * 4 * K))
    wkt_p = np.float32(wk).T.reshape(KC, 128, C4).transpose(1, 0, 2)
    wkt_p = wkt_p.reshape(128, 4 * C4)
    bv_rep = np.broadcast_to(np.float32(bv).reshape(1, C), (128, C))
    bq_c = np.float32(bq).reshape(C4, 1)
    bk_c = np.float32(bk).reshape(C4, 1)
    pk_n = [
        np.concatenate([wq_p, yt_p[i], wkt_p, bv_rep, bq_c, bk_c], axis=1)
        .astype(f16)
        for i in range(N)
    ]
    wvp = (np.float32(wv).T.reshape(KC, 128, C).transpose(1, 0, 2)
           .reshape(128, KC * C).astype(f16))
    s = np.ascontiguousarray(scale, dtype=np.float32).reshape(1, 1)
    return [
        {
            "x16": np.ascontiguousarray(x16[i]), "pk": pk_n[i], "wvp": wvp,
            "s": s,
        }
        for i in range(N)
    ]


def _run(inputs, **kwargs):
    nc = _build()
    return run_bass_kernel_spmd(nc, _in_maps(**inputs),
                                core_ids=list(range(N)), **kwargs)


def kernel(**inputs) -> np.ndarray:
    res = _run(inputs)
    # out16 [128, NPC*CT*PC] fp16 partition-major -> (C, HW) fp32
    out = np.stack([
        res.results[i]["out16"]
        .reshape(128, NPC, CT, PC)
        .transpose(2, 0, 1, 3)
        .reshape(C, HW)
        for i in range(N)
    ]).astype(np.float32)
    return out.reshape(N, C, H, W)
